# revision 5
# baseline (speedup 1.0000x reference)
"""Cross-attention kernel for Trainium2 (8 NeuronCores, Bass/Tile).

Problem: nn_CrossAttention — B=4, C=256, H=W=64 (N=4096 tokens), CI=128.
  q = q_w @ x + q_b            [B, N, CI]
  k = k_w @ rgbd + k_b         [B, CI, N]
  v = v_w @ rgbd + v_b         [B, N, CI]
  out = rgbd + out_w @ (softmax(q k) v) + out_b

Sharding: data-parallel over batch x query-half. Core i handles batch i//2,
query half i%2 (2048 queries, all 4096 keys). No collectives needed.

Math simplifications (exact):
  - k_b drops out of softmax (adds a per-query constant to logits).
  - v_b commutes with the softmax average: out_w @ (O + v_b) + out_b
    = out_w @ O + (out_w @ v_b + out_b)  -> fused output bias (host).
  - exp() without max-subtraction: logits are bounded (|S| <~ 45), safe in fp32.

On-core layout (all matmuls float32r, free dim 512 => 1 cycle/row):
  QT [CI,2048] = q_wT.T @ xs   (natural: CI on partitions)
  K  [CI,4096] = k_wT.T @ rs
  VT [CI,4096] = v_wT.T @ rs, then PE-transposed to V [k,CI] chunks
  per q-chunk (512):
    ST[k,q] tiles = K_chunk.T @ QT  (4 tiles batched per PSUM group)
      -> one exp ACTIVATE per 4 tiles (amortizes ACT's 352-cycle overhead)
    O_unnorm^T [CI,q] = sum_k V_chunk.T @ ET_chunk       (PSUM accum)
    d partials: 4-way column-packed ones-matmuls, tile_position (0,32g)
      (4 concurrent M=32 streams; then one (1/32)-matmul folds the 128
       partial rows into d[q] broadcast across partitions)
    O^T = O_unnorm^T * reciprocal(d);  out = out_wT.T @ O^T + ob + rgbd_slice
"""

import numpy as np

B, C, HH, WW = 4, 256, 64, 64
CI = 128
N = HH * WW            # 4096 tokens per batch
NCORES = 8
QSH = N // 2           # 2048 queries per core
QCH = 512              # query chunk (matmul moving free dim)
NQC = QSH // QCH       # 4 q-chunks
NKC = N // 128         # 32 key chunks of 128
EB = 4                 # S tiles per exp batch (PSUM-sourced ACTIVATE)
KQ = 8                 # key chunks per ET quarter-buffer
NQUARTER = NKC // KQ   # 4 quarters

_CACHE = {}


def _build_nc():
    import concourse.bass as bass
    import concourse.mybir as mybir
    import concourse.tile as tile
    from concourse import bacc
    from concourse.bass import ts
    from concourse.masks import make_identity

    f32 = mybir.dt.float32
    f32r = mybir.dt.float32r
    bf16 = mybir.dt.bfloat16
    EXP = mybir.ActivationFunctionType.Exp

    nc = bacc.Bacc("TRN2", target_bir_lowering=False, debug=False)

    rs_d = nc.dram_tensor("rs", [C, N], f32r, kind="ExternalInput")
    xs_d = nc.dram_tensor("xs", [C, QSH], f32r, kind="ExternalInput")
    res_d = nc.dram_tensor("res", [C, QSH], f32, kind="ExternalInput")
    qw_d = nc.dram_tensor("qw", [128, 2, 128], f32r, kind="ExternalInput")
    kw_d = nc.dram_tensor("kw", [128, 2, 128], f32r, kind="ExternalInput")
    vw_d = nc.dram_tensor("vw", [128, 2, 128], f32r, kind="ExternalInput")
    ow_d = nc.dram_tensor("ow", [128, C], f32r, kind="ExternalInput")
    qb_d = nc.dram_tensor("qb", [128, 1], f32, kind="ExternalInput")
    ob_d = nc.dram_tensor("ob", [128, 2], f32, kind="ExternalInput")
    out_d = nc.dram_tensor("out", [C, QSH], f32, kind="ExternalOutput")

    rs_r = rs_d.ap().rearrange("(co ci) n -> ci co n", ci=128)
    xs_r = xs_d.ap().rearrange("(co ci) n -> ci co n", ci=128)
    res_r = res_d.ap().rearrange("(co ci) n -> ci co n", ci=128)
    out_r = out_d.ap().rearrange("(co ci) n -> ci co n", ci=128)

    with tile.TileContext(nc) as tc:
        with (
            tc.tile_pool(name="const", bufs=1) as cpool,
            tc.tile_pool(name="big", bufs=2) as bigpool,
            tc.tile_pool(name="work", bufs=2) as wpool,
            tc.tile_pool(name="ps_s", bufs=1, space=bass.MemorySpace.PSUM) as ps_s,
            tc.tile_pool(name="ps_d", bufs=1, space=bass.MemorySpace.PSUM) as ps_d,
            tc.tile_pool(name="ps_o", bufs=2, space=bass.MemorySpace.PSUM) as ps_o,
            tc.tile_pool(name="ps_f", bufs=1, space=bass.MemorySpace.PSUM) as ps_f,
        ):
            # ---- weights / constants ----
            qw_sb = cpool.tile([128, 2, 128], f32r, tag="qw")
            kw_sb = cpool.tile([128, 2, 128], f32r, tag="kw")
            vw_sb = cpool.tile([128, 2, 128], f32r, tag="vw")
            ow_sb = cpool.tile([128, C], f32r, tag="ow")
            qb_sb = cpool.tile([128, 1], f32, tag="qb")
            ob_sb = cpool.tile([128, 2], f32, tag="ob")
            nc.sync.dma_start(qw_sb[:], qw_d.ap())
            nc.sync.dma_start(kw_sb[:], kw_d.ap())
            nc.sync.dma_start(vw_sb[:], vw_d.ap())
            nc.sync.dma_start(ow_sb[:], ow_d.ap())
            nc.sync.dma_start(qb_sb[:], qb_d.ap())
            nc.sync.dma_start(ob_sb[:], ob_d.ap())

            # ones32: 4-way packed denominator partials; onesR: 1/32 folds the
            # 128 partial rows (4 groups x 32 identical rows) back into d.
            ones_f = cpool.tile([128, 128], f32, tag="ones_f")
            nc.vector.memset(ones_f[:], 1.0)
            ones32 = cpool.tile([128, 32], bf16, tag="ones32")
            nc.vector.tensor_copy(ones32[:], ones_f[:, :32])
            onesR = cpool.tile([128, 128], f32r, tag="onesR")
            nc.vector.tensor_scalar_mul(onesR[:], ones_f[:], 1.0 / 32.0)
            id_f = cpool.tile([128, 128], f32, tag="ident_f")
            make_identity(nc, id_f[:])
            id_sb = cpool.tile([128, 128], f32r, tag="ident")
            nc.vector.tensor_copy(id_sb[:], id_f[:])

            # ---- activations: [C, n] laid out as [128, 2, n] (c = co*128 + ci) ----
            rs_sb = cpool.tile([128, 2, N], f32r, tag="rs")
            xs_sb = cpool.tile([128, 2, QSH], f32r, tag="xs")
            res_sb = cpool.tile([128, 2, QSH], f32, tag="res")

            K_sb = cpool.tile([128, N], f32r, tag="K")
            QT_sb = cpool.tile([128, QSH], f32r, tag="QT")
            V_sb = cpool.tile([128, NKC, 128], bf16, tag="V")

            # ---- K = k_wT.T @ rs (chunked DMA so projections start early) ----
            for j in range(N // 512):
                nc.sync.dma_start(rs_sb[:, :, ts(j, 512)], rs_r[:, :, ts(j, 512)])
            for j in range(QSH // 512):
                nc.sync.dma_start(xs_sb[:, :, ts(j, 512)], xs_r[:, :, ts(j, 512)])
            for j in range(N // 512):
                ps = ps_o.tile([128, 512], f32, tag="ops")
                for co in range(2):
                    nc.tensor.matmul(
                        ps[:],
                        kw_sb[:, co, :],
                        rs_sb[:, co, ts(j, 512)],
                        start=(co == 0),
                        stop=(co == 1),
                    )
                nc.vector.tensor_copy(K_sb[:, ts(j, 512)], ps[:])

            # ---- QT = q_wT.T @ xs + q_b ----
            for j in range(QSH // 512):
                ps = ps_o.tile([128, 512], f32, tag="ops")
                for co in range(2):
                    nc.tensor.matmul(
                        ps[:],
                        qw_sb[:, co, :],
                        xs_sb[:, co, ts(j, 512)],
                        start=(co == 0),
                        stop=(co == 1),
                    )
                nc.vector.tensor_scalar_add(QT_sb[:, ts(j, 512)], ps[:], qb_sb[:])

            # ---- VT = v_wT.T @ rs, then transpose to V[k, ci] chunks ----
            vt_sb = bigpool.tile([128, N], f32r, tag="big")
            for j in range(N // 512):
                ps = ps_o.tile([128, 512], f32, tag="ops")
                for co in range(2):
                    nc.tensor.matmul(
                        ps[:],
                        vw_sb[:, co, :],
                        rs_sb[:, co, ts(j, 512)],
                        start=(co == 0),
                        stop=(co == 1),
                    )
                nc.vector.tensor_copy(vt_sb[:, ts(j, 512)], ps[:])
            for kc in range(NKC):
                tps = ps_o.tile([128, 512], f32r, tag="ops")
                nc.tensor.transpose(tps[:, :128], vt_sb[:, ts(kc, 128)], id_sb[:])
                nc.vector.tensor_copy(V_sb[:, kc, :], tps[:, :128])

            # ---- main flash loop over query chunks ----
            for qc in range(NQC):
                qsl = ts(qc, QCH)
                nc.sync.dma_start(res_sb[:, :, qsl], res_r[:, :, qsl])

                dps = ps_d.tile([128, QCH], f32, tag="dps")
                ops = ps_o.tile([128, QCH], f32, tag="ops")
                for qq in range(NQUARTER):
                    et = bigpool.tile([128, KQ, QCH], bf16, tag="big")
                    # logits^T in batches of EB tiles -> one exp per batch
                    for bb in range(KQ // EB):
                        sps = ps_s.tile([128, EB, QCH], f32, tag="sps")
                        for i in range(EB):
                            kc = qq * KQ + bb * EB + i
                            nc.tensor.matmul(
                                sps[:, i, :],
                                K_sb[:, ts(kc, 128)],
                                QT_sb[:, qsl],
                            )
                        nc.scalar.activation(
                            et[:, ts(bb, EB), :], sps[:], EXP
                        )
                    # O_unnorm^T accumulation (full-width matmuls)
                    for i in range(KQ):
                        kc = qq * KQ + i
                        nc.tensor.matmul(
                            ops[:],
                            V_sb[:, kc, :],
                            et[:, i, :],
                            start=(kc == 0),
                            stop=(kc == NKC - 1),
                            skip_group_check=True,
                        )
                    # denominator partials: 4-way column-packed M=32 matmuls
                    # (consecutive instructions hit disjoint col groups ->
                    # they stream concurrently through separate XBUSes)
                    for i in range(KQ):
                        kc = qq * KQ + i
                        g = kc % 4
                        nc.tensor.matmul(
                            dps[32 * g : 32 * (g + 1), :],
                            ones32[:],
                            et[:, i, :],
                            start=(kc < 4),
                            stop=(kc >= NKC - 4),
                            skip_group_check=True,
                            tile_position=(0, 32 * g),
                        )

                # fold the 4 partial-row groups into d[q] (broadcast rows)
                d_part = wpool.tile([128, QCH], f32r, tag="dpart")
                nc.vector.tensor_copy(d_part[:], dps[:])
                drps = ps_f.tile([128, QCH], f32, tag="fps")
                nc.tensor.matmul(drps[:], onesR[:], d_part[:])
                rec = wpool.tile([128, QCH], f32, tag="rec")
                nc.vector.reciprocal(rec[:], drps[:])
                onorm = wpool.tile([128, QCH], f32r, tag="onorm")
                nc.vector.tensor_mul(onorm[:], ops[:], rec[:])

                # out[c_out, q] = out_wT.T @ O^T + ob + rgbd   (2 c_out tiles)
                ot = wpool.tile([128, 2, QCH], f32, tag="ost")
                for t in range(2):
                    fps = ps_f.tile([128, QCH], f32, tag="fps")
                    nc.tensor.matmul(fps[:], ow_sb[:, ts(t, 128)], onorm[:])
                    nc.vector.tensor_scalar_add(
                        ot[:, t, :], fps[:], ob_sb[:, t : t + 1]
                    )
                    nc.vector.tensor_add(ot[:, t, :], ot[:, t, :], res_sb[:, t, qsl])
                nc.sync.dma_start(out_r[:, :, qsl], ot[:])

    nc.compile()
    return nc


def _get_nc():
    if "nc" not in _CACHE:
        _CACHE["nc"] = _build_nc()
    return _CACHE["nc"]


def make_in_maps(rgbd, x, q_w, q_b, k_w, k_b, v_w, v_b, out_w, out_b):
    """Host-side sharding + weight swizzles. Returns per-core input maps."""
    f = np.float32
    rgbd = np.asarray(rgbd, f)
    x = np.asarray(x, f)
    q_w = np.asarray(q_w, f)
    q_b = np.asarray(q_b, f)
    k_w = np.asarray(k_w, f)
    v_w = np.asarray(v_w, f)
    out_w = np.asarray(out_w, f)
    out_b = np.asarray(out_b, f)
    v_b = np.asarray(v_b, f)

    # [ci_in, co, m] = w[m, co*128 + ci_in]
    def swz(w):
        return np.ascontiguousarray(w.reshape(CI, 2, 128).transpose(2, 1, 0))

    qw_sw, kw_sw, vw_sw = swz(q_w), swz(k_w), swz(v_w)
    ow_sw = np.ascontiguousarray(out_w.T)                    # [CI, C]
    qb_sw = np.ascontiguousarray(q_b.reshape(CI, 1))
    ob_fused = out_b + out_w @ v_b                           # [C]
    ob_sw = np.ascontiguousarray(ob_fused.reshape(2, 128).T)  # [128, 2]

    rs_all = rgbd.reshape(B, C, N)
    xs_all = x.reshape(B, C, N)

    in_maps = []
    for core in range(NCORES):
        b, h = divmod(core, 2)
        sl = slice(h * QSH, (h + 1) * QSH)
        in_maps.append(
            {
                "rs": np.ascontiguousarray(rs_all[b]),
                "xs": np.ascontiguousarray(xs_all[b][:, sl]),
                "res": np.ascontiguousarray(rs_all[b][:, sl]),
                "qw": qw_sw,
                "kw": kw_sw,
                "vw": vw_sw,
                "ow": ow_sw,
                "qb": qb_sw,
                "ob": ob_sw,
            }
        )
    return in_maps


def gather_out(results):
    out = np.empty((B, C, N), np.float32)
    for core in range(NCORES):
        b, h = divmod(core, 2)
        out[b][:, h * QSH : (h + 1) * QSH] = results[core]["out"]
    return out.reshape(B, C, HH, WW)


def kernel(**inputs):
    from concourse.bass_utils import run_bass_kernel_spmd

    in_maps = make_in_maps(**inputs)
    nc = _get_nc()
    res = run_bass_kernel_spmd(nc, in_maps, list(range(NCORES)))
    return gather_out(res.results)


# revision 6
# speedup vs baseline: 1.2240x; 1.2240x over previous
"""Cross-attention kernel for Trainium2 (8 NeuronCores, Bass/Tile).

Problem: nn_CrossAttention — B=4, C=256, H=W=64 (N=4096 tokens), CI=128.
  q = q_w @ x + q_b            [B, N, CI]
  k = k_w @ rgbd + k_b         [B, CI, N]
  v = v_w @ rgbd + v_b         [B, N, CI]
  out = rgbd + out_w @ (softmax(q k) v) + out_b

Sharding: data-parallel over batch x query-half. Core i handles batch i//2,
query half i%2 (2048 queries, all 4096 keys). No collectives needed.

Math simplifications (exact):
  - k_b drops out of softmax (adds a per-query constant to logits).
  - v_b commutes with the softmax average: out_w @ (O + v_b) + out_b
    = out_w @ O + (out_w @ v_b + out_b)  -> fused output bias (host).
  - exp() without max-subtraction: logits are bounded (|S| <~ 45), safe in fp32.

Engine plan (per core):
  - Main matmuls (S, AV, denominator) in bf16: 1 cycle/row + FWL weight loads
    (fp32 weights disable FWL and serialize ~150ns/matmul).  Logit/psum
    accumulation is always fp32 in PSUM; residual + biases applied in fp32.
  - ST[k,q] = K_chunk.T @ QT batches 2 tiles/PSUM group, double-buffered, one
    exp ACTIVATE per batch (amortizes ScalarE's ~350-cycle fixed overhead).
  - O_unnorm^T accumulates across 32 key chunks in one PSUM bank.
  - Denominator: 4-way column-packed M=32 ones-matmuls (tile_position
    (0,32g) -> 4 concurrent streams), then a (1/32)-ones f32r matmul folds
    the 128 partial rows into d[q] broadcast across all partitions.
  - O^T = O_unnorm^T * reciprocal(d); out = out_wT.T @ O^T + ob + rgbd_slice.
"""

import numpy as np

B, C, HH, WW = 4, 256, 64, 64
CI = 128
N = HH * WW            # 4096 tokens per batch
NCORES = 8
QSH = N // 2           # 2048 queries per core
QCH = 512              # query chunk (matmul moving free dim)
NQC = QSH // QCH       # 4 q-chunks
NKC = N // 128         # 32 key chunks of 128
EB = 2                 # S tiles per exp batch (PSUM-sourced ACTIVATE)
KQ = 8                 # key chunks per ET quarter-buffer
NQUARTER = NKC // KQ   # 4 quarters

_CACHE = {}


def _build_nc():
    import concourse.bass as bass
    import concourse.mybir as mybir
    import concourse.tile as tile
    from concourse import bacc
    from concourse.bass import ts
    from concourse.masks import make_identity

    f32 = mybir.dt.float32
    f32r = mybir.dt.float32r
    bf16 = mybir.dt.bfloat16
    EXP = mybir.ActivationFunctionType.Exp

    nc = bacc.Bacc("TRN2", target_bir_lowering=False, debug=False)

    rs_d = nc.dram_tensor("rs", [C, N], bf16, kind="ExternalInput")
    xs_d = nc.dram_tensor("xs", [C, QSH], bf16, kind="ExternalInput")
    res_d = nc.dram_tensor("res", [C, QSH], f32, kind="ExternalInput")
    qw_d = nc.dram_tensor("qw", [128, 2, 128], bf16, kind="ExternalInput")
    kw_d = nc.dram_tensor("kw", [128, 2, 128], bf16, kind="ExternalInput")
    vw_d = nc.dram_tensor("vw", [128, 2, 128], bf16, kind="ExternalInput")
    ow_d = nc.dram_tensor("ow", [128, C], f32r, kind="ExternalInput")
    qb_d = nc.dram_tensor("qb", [128, 1], f32, kind="ExternalInput")
    ob_d = nc.dram_tensor("ob", [128, 2], f32, kind="ExternalInput")
    out_d = nc.dram_tensor("out", [C, QSH], f32, kind="ExternalOutput")

    rs_r = rs_d.ap().rearrange("(co ci) n -> ci co n", ci=128)
    xs_r = xs_d.ap().rearrange("(co ci) n -> ci co n", ci=128)
    res_r = res_d.ap().rearrange("(co ci) n -> ci co n", ci=128)
    out_r = out_d.ap().rearrange("(co ci) n -> ci co n", ci=128)

    with tile.TileContext(nc) as tc:
        with (
            tc.tile_pool(name="const", bufs=1) as cpool,
            tc.tile_pool(name="big", bufs=2) as bigpool,
            tc.tile_pool(name="work", bufs=2) as wpool,
            tc.tile_pool(name="ps_s", bufs=2, space=bass.MemorySpace.PSUM) as ps_s,
            tc.tile_pool(name="ps_d", bufs=1, space=bass.MemorySpace.PSUM) as ps_d,
            tc.tile_pool(name="ps_o", bufs=2, space=bass.MemorySpace.PSUM) as ps_o,
            tc.tile_pool(name="ps_f", bufs=1, space=bass.MemorySpace.PSUM) as ps_f,
        ):
            # ---- weights / constants ----
            qw_sb = cpool.tile([128, 2, 128], bf16, tag="qw")
            kw_sb = cpool.tile([128, 2, 128], bf16, tag="kw")
            vw_sb = cpool.tile([128, 2, 128], bf16, tag="vw")
            ow_sb = cpool.tile([128, C], f32r, tag="ow")
            qb_sb = cpool.tile([128, 1], f32, tag="qb")
            ob_sb = cpool.tile([128, 2], f32, tag="ob")
            nc.sync.dma_start(qw_sb[:], qw_d.ap())
            nc.sync.dma_start(kw_sb[:], kw_d.ap())
            nc.sync.dma_start(vw_sb[:], vw_d.ap())
            nc.sync.dma_start(ow_sb[:], ow_d.ap())
            nc.sync.dma_start(qb_sb[:], qb_d.ap())
            nc.sync.dma_start(ob_sb[:], ob_d.ap())

            # ones32: 4-way packed denominator partials; onesR: 1/32 folds the
            # 128 partial rows (4 groups x 32 identical rows) back into d.
            ones_f = cpool.tile([128, 128], f32, tag="ones_f")
            nc.vector.memset(ones_f[:], 1.0)
            ones32 = cpool.tile([128, 32], bf16, tag="ones32")
            nc.vector.tensor_copy(ones32[:], ones_f[:, :32])
            onesR = cpool.tile([128, 128], f32r, tag="onesR")
            nc.vector.tensor_scalar_mul(onesR[:], ones_f[:], 1.0 / 32.0)
            id_f = cpool.tile([128, 128], f32, tag="ident_f")
            make_identity(nc, id_f[:])
            id_sb = cpool.tile([128, 128], bf16, tag="ident")
            nc.vector.tensor_copy(id_sb[:], id_f[:])

            # warm the ScalarE exp table while projections run
            warm = cpool.tile([128, 1], f32, tag="warm")
            nc.scalar.activation(warm[:], ones_f[:, :1], EXP)

            # ---- activations: [C, n] laid out as [128, 2, n] (c = co*128 + ci) ----
            rs_sb = cpool.tile([128, 2, N], bf16, tag="rs")
            xs_sb = cpool.tile([128, 2, QSH], bf16, tag="xs")
            res_sb = cpool.tile([128, 2, QSH], f32, tag="res")

            K_sb = cpool.tile([128, N], bf16, tag="K")
            QT_sb = cpool.tile([128, QSH], bf16, tag="QT")
            V_sb = cpool.tile([128, NKC, 128], bf16, tag="V")

            # ---- K = k_wT.T @ rs (chunked DMA so projections start early) ----
            for j in range(N // 512):
                nc.sync.dma_start(rs_sb[:, :, ts(j, 512)], rs_r[:, :, ts(j, 512)])
            for j in range(QSH // 512):
                nc.sync.dma_start(xs_sb[:, :, ts(j, 512)], xs_r[:, :, ts(j, 512)])
            for j in range(N // 512):
                ps = ps_o.tile([128, 512], f32, tag="ops")
                for co in range(2):
                    nc.tensor.matmul(
                        ps[:],
                        kw_sb[:, co, :],
                        rs_sb[:, co, ts(j, 512)],
                        start=(co == 0),
                        stop=(co == 1),
                    )
                nc.vector.tensor_copy(K_sb[:, ts(j, 512)], ps[:])

            # ---- QT = q_wT.T @ xs + q_b ----
            for j in range(QSH // 512):
                ps = ps_o.tile([128, 512], f32, tag="ops")
                for co in range(2):
                    nc.tensor.matmul(
                        ps[:],
                        qw_sb[:, co, :],
                        xs_sb[:, co, ts(j, 512)],
                        start=(co == 0),
                        stop=(co == 1),
                    )
                nc.vector.tensor_scalar_add(QT_sb[:, ts(j, 512)], ps[:], qb_sb[:])

            # ---- VT = v_wT.T @ rs, then transpose to V[k, ci] chunks ----
            vt_sb = bigpool.tile([128, N], bf16, tag="big")
            for j in range(N // 512):
                ps = ps_o.tile([128, 512], f32, tag="ops")
                for co in range(2):
                    nc.tensor.matmul(
                        ps[:],
                        vw_sb[:, co, :],
                        rs_sb[:, co, ts(j, 512)],
                        start=(co == 0),
                        stop=(co == 1),
                    )
                nc.vector.tensor_copy(vt_sb[:, ts(j, 512)], ps[:])
            for kc in range(NKC):
                tps = ps_o.tile([128, 512], bf16, tag="ops")
                nc.tensor.transpose(tps[:, :128], vt_sb[:, ts(kc, 128)], id_sb[:])
                nc.vector.tensor_copy(V_sb[:, kc, :], tps[:, :128])

            # ---- main flash loop over query chunks ----
            for qc in range(NQC):
                qsl = ts(qc, QCH)
                nc.sync.dma_start(res_sb[:, :, qsl], res_r[:, :, qsl])

                dps = ps_d.tile([128, QCH], f32, tag="dps")
                ops = ps_o.tile([128, QCH], f32, tag="ops")
                for qq in range(NQUARTER):
                    et = bigpool.tile([128, KQ, QCH], bf16, tag="big")
                    # logits^T in batches of EB tiles -> one exp per batch
                    for bb in range(KQ // EB):
                        sps = ps_s.tile([128, EB, QCH], f32, tag="sps")
                        for i in range(EB):
                            kc = qq * KQ + bb * EB + i
                            nc.tensor.matmul(
                                sps[:, i, :],
                                K_sb[:, ts(kc, 128)],
                                QT_sb[:, qsl],
                            )
                        nc.scalar.activation(et[:, ts(bb, EB), :], sps[:], EXP)
                    # O_unnorm^T accumulation (full-width matmuls)
                    for i in range(KQ):
                        kc = qq * KQ + i
                        nc.tensor.matmul(
                            ops[:],
                            V_sb[:, kc, :],
                            et[:, i, :],
                            start=(kc == 0),
                            stop=(kc == NKC - 1),
                            skip_group_check=True,
                        )
                    # denominator partials: 4-way column-packed M=32 matmuls
                    # (consecutive instructions hit disjoint col groups ->
                    # they stream concurrently through separate XBUSes)
                    for i in range(KQ):
                        kc = qq * KQ + i
                        g = kc % 4
                        nc.tensor.matmul(
                            dps[32 * g : 32 * (g + 1), :],
                            ones32[:],
                            et[:, i, :],
                            start=(kc < 4),
                            stop=(kc >= NKC - 4),
                            skip_group_check=True,
                            tile_position=(0, 32 * g),
                        )

                # fold the 4 partial-row groups into d[q] (broadcast rows)
                d_part = wpool.tile([128, QCH], f32r, tag="dpart")
                nc.vector.tensor_copy(d_part[:], dps[:])
                drps = ps_f.tile([128, QCH], f32, tag="fps")
                nc.tensor.matmul(drps[:], onesR[:], d_part[:])
                rec = wpool.tile([128, QCH], f32, tag="rec")
                nc.vector.reciprocal(rec[:], drps[:])
                onorm = wpool.tile([128, QCH], f32r, tag="onorm")
                nc.vector.tensor_mul(onorm[:], ops[:], rec[:])

                # out[c_out, q] = out_wT.T @ O^T + ob + rgbd   (2 c_out tiles)
                ot = wpool.tile([128, 2, QCH], f32, tag="ost")
                for t in range(2):
                    fps = ps_f.tile([128, QCH], f32, tag="fps")
                    nc.tensor.matmul(fps[:], ow_sb[:, ts(t, 128)], onorm[:])
                    nc.vector.tensor_scalar_add(
                        ot[:, t, :], fps[:], ob_sb[:, t : t + 1]
                    )
                    nc.vector.tensor_add(ot[:, t, :], ot[:, t, :], res_sb[:, t, qsl])
                nc.sync.dma_start(out_r[:, :, qsl], ot[:])

    nc.compile()
    return nc


def _get_nc():
    if "nc" not in _CACHE:
        _CACHE["nc"] = _build_nc()
    return _CACHE["nc"]


def make_in_maps(rgbd, x, q_w, q_b, k_w, k_b, v_w, v_b, out_w, out_b):
    """Host-side sharding + weight swizzles. Returns per-core input maps."""
    import ml_dtypes

    f = np.float32
    bf = ml_dtypes.bfloat16
    rgbd = np.asarray(rgbd, f)
    x = np.asarray(x, f)
    q_w = np.asarray(q_w, f)
    q_b = np.asarray(q_b, f)
    k_w = np.asarray(k_w, f)
    v_w = np.asarray(v_w, f)
    out_w = np.asarray(out_w, f)
    out_b = np.asarray(out_b, f)
    v_b = np.asarray(v_b, f)

    # [ci_in, co, m] = w[m, co*128 + ci_in]
    def swz(w):
        return np.ascontiguousarray(
            w.reshape(CI, 2, 128).transpose(2, 1, 0).astype(bf)
        )

    qw_sw, kw_sw, vw_sw = swz(q_w), swz(k_w), swz(v_w)
    ow_sw = np.ascontiguousarray(out_w.T)                    # [CI, C] f32
    qb_sw = np.ascontiguousarray(q_b.reshape(CI, 1))
    ob_fused = out_b + out_w @ v_b                           # [C]
    ob_sw = np.ascontiguousarray(ob_fused.reshape(2, 128).T)  # [128, 2]

    rs_all = rgbd.reshape(B, C, N)
    xs_all = x.reshape(B, C, N)

    in_maps = []
    for core in range(NCORES):
        b, h = divmod(core, 2)
        sl = slice(h * QSH, (h + 1) * QSH)
        in_maps.append(
            {
                "rs": np.ascontiguousarray(rs_all[b].astype(bf)),
                "xs": np.ascontiguousarray(xs_all[b][:, sl].astype(bf)),
                "res": np.ascontiguousarray(rs_all[b][:, sl]),
                "qw": qw_sw,
                "kw": kw_sw,
                "vw": vw_sw,
                "ow": ow_sw,
                "qb": qb_sw,
                "ob": ob_sw,
            }
        )
    return in_maps


def gather_out(results):
    out = np.empty((B, C, N), np.float32)
    for core in range(NCORES):
        b, h = divmod(core, 2)
        out[b][:, h * QSH : (h + 1) * QSH] = results[core]["out"]
    return out.reshape(B, C, HH, WW)


def kernel(**inputs):
    from concourse.bass_utils import run_bass_kernel_spmd

    in_maps = make_in_maps(**inputs)
    nc = _get_nc()
    res = run_bass_kernel_spmd(nc, in_maps, list(range(NCORES)))
    return gather_out(res.results)
